# revision 48
# baseline (speedup 1.0000x reference)
"""Causal self-attention for trn2, 8 NeuronCores.

Problem: x[4,2048,1024] @ w_qkv[1024,3072] -> causal MHA (16 heads, d=64)
-> @ w_out[1024,1024].

Sharding: core c handles batch b=c%4 and heads hbase=8*(c//4)..hbase+8
(data parallel on B x tensor parallel on heads). Each core computes the
partial out-projection y_c = att_slice @ w_out[slice]; the host sums the
two partials per batch.

v5: restructured from v4 for TensorE saturation.
- All qkv projections run up-front (per quarter), with qT for every
  quarter retained in SBUF. This front-loads TensorE work so the
  scheduler can fill exp-paced attention gaps with projection matmuls.
- Scores for the two heads of a group are row-tiled (K=64 each,
  tile_position (0,0)/(64,0)) and run concurrently into one [128,1024]
  PSUM pair; a single exp covers both heads.
- Softmax denominators come from the fused ones-column in AV (row 64 of
  the [65,512] accumulators). Normalization scatters both denominator
  rows through DRAM into a [128,8] tile so the reciprocal runs on 128
  DVE lanes (~0.1us) instead of one (3.3us), then DMA-broadcasts back.
- Diagonal k-tiles only exp the causal columns (memset the rest).
- PSUM: sc [128,1024]x2 (4 banks) + av0/av1 [65,512] (2 banks) +
  pj [128,512]x2 (2 banks) shared by qkv-proj, V-proj and out-proj.
"""

import sys

for p in ("/opt/trn_rl_repo", "/opt/pypackages"):
    if p not in sys.path:
        sys.path.insert(0, p)

import contextlib

import numpy as np

import concourse.bass as bass
import concourse.mybir as mybir
import concourse.tile as tile
from concourse import bacc
from concourse.bass_utils import run_bass_kernel_spmd
from concourse.masks import make_identity

F32 = mybir.dt.float32
BF = mybir.dt.bfloat16
EXP = mybir.ActivationFunctionType.Exp

T = 2048          # sequence length
C = 1024          # model dim
HC = 8            # heads per core
D = 64            # head dim
NG = 4            # head-groups of 2 per core
NCT = C // 128    # 8 contraction tiles
NTT = T // 128    # 16 token tiles
NQ = 4            # T quarters
SCALE = 0.125     # 1/sqrt(D)


def build_nc():
    nc = bacc.Bacc("TRN2", target_bir_lowering=False, debug=False)

    # All matmul operands are bf16 anyway, so inputs arrive pre-cast to
    # bf16 from the host: halves the startup DMA bytes and removes every
    # staging cast (device cast via DVE would be identical numerics).
    # x additionally arrives PRE-TRANSPOSED ([C, T]) so the xT operand the
    # projections need loads with plain DMAs - no on-chip transposes.
    x_d = nc.dram_tensor("x", [C, T], BF, kind="ExternalInput")
    wq_d = nc.dram_tensor("wq", [C, 512], BF, kind="ExternalInput")
    wk_d = nc.dram_tensor("wk", [C, 512], BF, kind="ExternalInput")
    wv_d = nc.dram_tensor("wv", [C, 512], BF, kind="ExternalInput")
    wo_d = nc.dram_tensor("wo", [512, C], BF, kind="ExternalInput")
    y_d = nc.dram_tensor("y", [T, C], F32, kind="ExternalOutput")

    with tile.TileContext(nc) as tc, contextlib.ExitStack() as ctx:
        persist = ctx.enter_context(tc.tile_pool(name="persist", bufs=1))
        work = ctx.enter_context(tc.tile_pool(name="work", bufs=1))
        ps = ctx.enter_context(tc.tile_pool(name="ps", bufs=1, space="PSUM"))
        dpool = ctx.enter_context(tc.tile_pool(name="dram", bufs=1, space="DRAM"))

        kT = [persist.tile([128, T], BF, tag=f"kT{g}", name=f"kT{g}")
              for g in range(NG)]
        qT = [persist.tile([128, T], BF, tag=f"qT{g}", name=f"qT{g}")
              for g in range(NG)]
        V = persist.tile([128, NTT, HC, 65], BF, tag="V")

        # xT resident in SBUF: one tile PER QUARTER (separate tiles keep
        # the dependency tracking fine-grained, so quarter-0 projections
        # never wait on later quarters' loads).
        xT_q = [persist.tile([128, NCT, 512], BF, tag=f"xTq{rnd}",
                              name=f"xTq{rnd}")
                for rnd in range(NQ)]

        def load_xT(rnd):
            nc.sync.dma_start(
                out=xT_q[rnd],
                in_=x_d.ap()[:, rnd * 512:(rnd + 1) * 512].rearrange(
                    "(ct p) t -> p ct t", p=128))

        load_xT(0)

        # weights: bf16 loads on the scalar HWDGE queue; the sync queue
        # carries xT and the normalization bounce.
        wq_bf = persist.tile([128, NCT, 512], BF, tag="wq_bf")
        wk_bf = persist.tile([128, NCT, 512], BF, tag="wk_bf")
        wv_bf = persist.tile([128, NCT, 512], BF, tag="wv_bf")
        wo_bf = persist.tile([128, NG, C], BF, tag="wo_bf")
        for wdram, wbf in ((wq_d, wq_bf), (wv_d, wv_bf), (wk_d, wk_bf)):
            nc.scalar.dma_start(
                out=wbf, in_=wdram.ap().rearrange("(ct p) m -> p ct m", p=128))
        nc.scalar.dma_start(
            out=wo_bf, in_=wo_d.ap().rearrange("(g p) c -> p g c", p=128))

        # ones column of V (fused softmax denominator)
        ones_f32 = persist.tile([128, NTT, HC], F32, tag="ones")
        nc.vector.memset(ones_f32, 1.0)
        nc.vector.tensor_copy(V[:, :, :, 64], ones_f32)

        # DRAM scratch for the reciprocal broadcast bounce
        rrec_d = [dpool.tile([1, 1024], F32, tag=f"rrec{i}", name=f"rrec{i}",
                             bufs=2)
                  for i in range(NG)]

        def emit_outproj(qb, att):
            # out projection for quarter qb's q rows. Emitted AFTER the
            # next quarter's projections so the shared "pj" PSUM rotation
            # never makes projections wait on the normalization chain.
            for qtl in range(4):
                qt = qb * 4 + qtl
                y_sb = work.tile([128, C], F32, tag="y_sb", bufs=2,
                                 name="y_sb")
                for half in range(2):
                    psy = ps.tile([128, 512], F32, tag="pj", bufs=2,
                                  name="psy")
                    for g in range(NG):
                        nc.tensor.matmul(
                            psy,
                            att[g][:, qtl * 128:(qtl + 1) * 128],
                            wo_bf[:, g, half * 512:(half + 1) * 512],
                            start=(g == 0),
                            stop=(g == NG - 1),
                        )
                    nc.vector.tensor_copy(
                        y_sb[:, half * 512:(half + 1) * 512], psy)
                # y rides the otherwise-idle gpsimd SWDGE ring, except
                # the final quarter which uses the (by then idle) scalar
                # ring for lower tail latency.
                yeng = nc.scalar if qb == 3 else nc.gpsimd
                yeng.dma_start(
                    out=y_d.ap()[qt * 128:(qt + 1) * 128, :], in_=y_sb
                )

        att_q = {}
        for qb in range(NQ):
            q0 = qb * 512
            nkt = 4 * (qb + 1)
            if qb + 1 < NQ:
                load_xT(qb + 1)
            if qb == 3:
                # deferred out-projections: emitted BEFORE attention(q3)
                # so they hold interleaving priority and fill the
                # exp-paced gaps of the last quarter.
                for pqb in range(3):
                    emit_outproj(pqb, att_q[pqb])
            att = [work.tile([128, 512], BF, tag=f"att{g}", name=f"att{g}",
                             bufs=4)
                   for g in range(NG)]
            att_q[qb] = att
            for g in range(NG):
                # ---- qT/kT for group g ----
                for which, wbf, dst in ((0, wq_bf, qT[g]), (1, wk_bf, kT[g])):
                    pj = ps.tile([128, 512], F32, tag="pj", bufs=2, name="pj")
                    for ct in range(NCT):
                        nc.tensor.matmul(
                            pj,
                            wbf[:, ct, g * 128:(g + 1) * 128],
                            xT_q[qb][:, ct, :],
                            start=(ct == 0), stop=(ct == NCT - 1),
                        )
                    nc.vector.tensor_copy(dst[:, q0:q0 + 512], pj)

                if g == 0:
                    # ---- V for this quarter: must be emitted BEFORE the
                    # attention below (program order defines dataflow; the
                    # diagonal AVs read this quarter's V) ----
                    for tt in range(4):
                        pv = ps.tile([128, HC, 64], F32, tag="pj", bufs=2,
                                     name="pv")
                        for ct in range(NCT):
                            nc.tensor.matmul(
                                pv,
                                xT_q[qb][:, ct, tt * 128:(tt + 1) * 128],
                                wv_bf[:, ct, :],
                                start=(ct == 0), stop=(ct == NCT - 1),
                            )
                        nc.vector.tensor_copy(V[:, qb * 4 + tt, :, 0:64], pv)

                # ---- attention for group g ----
                av0 = ps.tile([65, 512], F32, tag="av0", name="av0")
                av1 = ps.tile([65, 512], F32, tag="av1", name="av1")
                for kt in range(nkt):
                    j = kt - 4 * qb
                    # diagonal block: in wT[:, col] (keys on partitions p,
                    # queries on cols) the keep condition is
                    # col - p - 128j >= 0, so cols [0,128j) are fully
                    # masked. Skip them in the scores matmul, the exp and
                    # the AV matmul; only [128j, 128j+128) needs the
                    # triangular select.
                    z = 128 * j if j > 0 else 0
                    sc = ps.tile([128, 1024], F32, tag="sc", bufs=2, name="sc")
                    for hh in range(2):
                        o = hh * 512
                        nc.tensor.matmul(
                            sc[:, o + z:o + 512],
                            kT[g][hh * 64:hh * 64 + 64,
                                  kt * 128:(kt + 1) * 128],
                            qT[g][hh * 64:hh * 64 + 64,
                                  q0 + z:q0 + 512],
                            start=True, stop=True,
                            tile_position=(64 * hh, 0),
                        )
                    wT = work.tile([128, 1024], BF, tag="wT", bufs=4)
                    if j >= 2:
                        # exp only the live columns (2-instruction split
                        # only pays off once >=256 cols are masked)
                        for hh in range(2):
                            o = hh * 512
                            nc.vector.memset(wT[:, o:o + z], 0.0)
                            nc.scalar.activation(
                                wT[:, o + z:o + 512], sc[:, o + z:o + 512],
                                EXP, scale=SCALE)
                            nc.gpsimd.affine_select(
                                out=wT[:, o + z:o + z + 128],
                                in_=wT[:, o + z:o + z + 128],
                                compare_op=mybir.AluOpType.is_ge,
                                fill=0.0,
                                base=0,
                                pattern=[[1, 128]],
                                channel_multiplier=-1,
                            )
                    elif j >= 0:
                        # full-width exp (cols [0,128j) hold stale psum,
                        # exp'd then zeroed by the select below)
                        nc.scalar.activation(wT, sc, EXP, scale=SCALE)
                        ncols = 128 * (j + 1)
                        for hh in range(2):
                            o = hh * 512
                            nc.gpsimd.affine_select(
                                out=wT[:, o:o + ncols],
                                in_=wT[:, o:o + ncols],
                                compare_op=mybir.AluOpType.is_ge,
                                fill=0.0,
                                base=-z,
                                pattern=[[1, ncols]],
                                channel_multiplier=-1,
                            )
                    else:
                        nc.scalar.activation(wT, sc, EXP, scale=SCALE)
                    for hh, av in ((0, av0), (1, av1)):
                        nc.tensor.matmul(
                            av[:, z:512], V[:, kt, 2 * g + hh, :],
                            wT[:, hh * 512 + z:(hh + 1) * 512],
                            start=(kt == 0), stop=(kt == nkt - 1),
                        )

                # ---- normalize: denominators via [128,8] reciprocal ----
                avc = work.tile([65, 1024], F32, tag="avc", bufs=2,
                                name="avc")
                nc.vector.tensor_copy(avc[:, 0:512], av0)
                nc.vector.tensor_copy(avc[:, 512:1024], av1)
                # SBUF->SBUF partition scatter of the denominator row so
                # the reciprocal runs on all 128 DVE lanes.
                # the very last group's chain runs after all exps are done,
                # so it can use the scalar ring and skip the sync-ring
                # backlog of the earlier groups' chains.
                neng = nc.scalar if (qb == 3 and g == 3) else nc.sync
                dsc = work.tile([128, 8], F32, tag="dsc", bufs=2, name="dsc")
                neng.dma_start(out=dsc, in_=avc[64:65, :])
                rec = work.tile([128, 8], F32, tag="rec", bufs=2, name="rec")
                nc.vector.reciprocal(rec, dsc)
                # gather back to DRAM in q-major order per head, then one
                # stride-0 broadcast read for both heads.
                neng.dma_start(
                    out=bass.AP(rrec_d[g].tensor, rrec_d[g].offset,
                                [[8, 128], [1, 8]]),
                    in_=rec)
                rep = work.tile([64, 1024], F32, tag="rep", bufs=2,
                                name="rep")
                neng.dma_start(
                    out=rep,
                    in_=bass.AP(rrec_d[g].tensor, rrec_d[g].offset,
                                [[0, 64], [1, 1024]]))
                nc.vector.tensor_mul(att[g][0:64, :], avc[0:64, 0:512],
                                     rep[:, 0:512])
                tmpB = work.tile([64, 512], BF, tag="tmpB", bufs=2,
                                 name="tmpB")
                nc.vector.tensor_mul(tmpB, avc[0:64, 512:1024],
                                     rep[:, 512:1024])
                neng.dma_start(out=att[g][64:128, :], in_=tmpB)

        # quarter 3's out-projection forms the tail: use the sc PSUM
        # banks (free once the last scores ran) as paired accumulators so
        # two qtl are in flight and each drains with a single copy.
        for qtl in range(4):
            pspair = ps.tile([128, 1024], F32, tag="sc", bufs=2,
                             name="psyt")
            for half in range(2):
                for g in range(NG):
                    nc.tensor.matmul(
                        pspair[:, half * 512:(half + 1) * 512],
                        att_q[3][g][:, qtl * 128:(qtl + 1) * 128],
                        wo_bf[:, g, half * 512:(half + 1) * 512],
                        start=(g == 0), stop=(g == NG - 1),
                    )
            y_sb = work.tile([128, C], F32, tag="y_sb", bufs=2, name="y_sb")
            nc.vector.tensor_copy(y_sb, pspair)
            nc.scalar.dma_start(
                out=y_d.ap()[(12 + qtl) * 128:(13 + qtl) * 128, :], in_=y_sb)


    nc.compile()
    return nc


_NC_CACHE = None


def _get_nc():
    global _NC_CACHE
    if _NC_CACHE is None:
        _NC_CACHE = build_nc()
    return _NC_CACHE


def kernel(x, w_qkv, w_out, _trace=False):
    import ml_dtypes

    bf16 = ml_dtypes.bfloat16
    B = x.shape[0]
    x = np.asarray(x, dtype=np.float32).astype(bf16)
    w_qkv = np.asarray(w_qkv, dtype=np.float32).astype(bf16)
    w_out = np.asarray(w_out, dtype=np.float32).astype(bf16)

    nc = _get_nc()
    in_maps = []
    for core in range(8):
        b = core % B
        hbase = (core // B) * HC
        lo, hi = hbase * D, hbase * D + HC * D
        in_maps.append({
            "x": np.ascontiguousarray(x[b].T),
            "wq": np.ascontiguousarray(w_qkv[:, lo:hi]),
            "wk": np.ascontiguousarray(w_qkv[:, C + lo:C + hi]),
            "wv": np.ascontiguousarray(w_qkv[:, 2 * C + lo:2 * C + hi]),
            "wo": np.ascontiguousarray(w_out[lo:hi, :]),
        })

    res = run_bass_kernel_spmd(nc, in_maps, core_ids=list(range(8)), trace=_trace)
    ys = [r["y"] for r in res.results]
    out = np.empty((B, T, C), dtype=np.float32)
    for b in range(B):
        out[b] = ys[b] + ys[b + B]
    if _trace:
        return out, res
    return out


# revision 50
# speedup vs baseline: 1.0545x; 1.0545x over previous
"""Causal self-attention for trn2, 8 NeuronCores.

Problem: x[4,2048,1024] @ w_qkv[1024,3072] -> causal MHA (16 heads, d=64)
-> @ w_out[1024,1024].

Sharding: core c handles batch b=c%4 and heads hbase=8*(c//4)..hbase+8
(data parallel on B x tensor parallel on heads). Each core computes the
partial out-projection y_c = att_slice @ w_out[slice]; the host sums the
two partials per batch.

v5: restructured from v4 for TensorE saturation.
- All qkv projections run up-front (per quarter), with qT for every
  quarter retained in SBUF. This front-loads TensorE work so the
  scheduler can fill exp-paced attention gaps with projection matmuls.
- Scores for the two heads of a group are row-tiled (K=64 each,
  tile_position (0,0)/(64,0)) and run concurrently into one [128,1024]
  PSUM pair; a single exp covers both heads.
- Softmax denominators come from the fused ones-column in AV (row 64 of
  the [65,512] accumulators). Normalization scatters both denominator
  rows through DRAM into a [128,8] tile so the reciprocal runs on 128
  DVE lanes (~0.1us) instead of one (3.3us), then DMA-broadcasts back.
- Diagonal k-tiles only exp the causal columns (memset the rest).
- PSUM: sc [128,1024]x2 (4 banks) + av0/av1 [65,512] (2 banks) +
  pj [128,512]x2 (2 banks) shared by qkv-proj, V-proj and out-proj.
"""

import sys

for p in ("/opt/trn_rl_repo", "/opt/pypackages"):
    if p not in sys.path:
        sys.path.insert(0, p)

import contextlib

import numpy as np

import concourse.bass as bass
import concourse.mybir as mybir
import concourse.tile as tile
from concourse import bacc
from concourse.bass_utils import run_bass_kernel_spmd
from concourse.masks import make_identity

F32 = mybir.dt.float32
BF = mybir.dt.bfloat16
EXP = mybir.ActivationFunctionType.Exp

T = 2048          # sequence length
C = 1024          # model dim
HC = 8            # heads per core
D = 64            # head dim
NG = 4            # head-groups of 2 per core
NCT = C // 128    # 8 contraction tiles
NTT = T // 128    # 16 token tiles
NQ = 4            # T quarters
SCALE = 0.125     # 1/sqrt(D)


def build_nc():
    nc = bacc.Bacc("TRN2", target_bir_lowering=False, debug=False)

    # All matmul operands are bf16 anyway, so inputs arrive pre-cast to
    # bf16 from the host: halves the startup DMA bytes and removes every
    # staging cast (device cast via DVE would be identical numerics).
    # x additionally arrives PRE-TRANSPOSED ([C, T]) so the xT operand the
    # projections need loads with plain DMAs - no on-chip transposes.
    x_d = nc.dram_tensor("x", [C, T], BF, kind="ExternalInput")
    wq_d = nc.dram_tensor("wq", [C, 512], BF, kind="ExternalInput")
    wk_d = nc.dram_tensor("wk", [C, 512], BF, kind="ExternalInput")
    wv_d = nc.dram_tensor("wv", [C, 512], BF, kind="ExternalInput")
    wo_d = nc.dram_tensor("wo", [512, C], BF, kind="ExternalInput")
    y_d = nc.dram_tensor("y", [T, C], F32, kind="ExternalOutput")

    with tile.TileContext(nc) as tc, contextlib.ExitStack() as ctx:
        persist = ctx.enter_context(tc.tile_pool(name="persist", bufs=1))
        work = ctx.enter_context(tc.tile_pool(name="work", bufs=1))
        ps = ctx.enter_context(tc.tile_pool(name="ps", bufs=1, space="PSUM"))
        dpool = ctx.enter_context(tc.tile_pool(name="dram", bufs=1, space="DRAM"))

        kT = [persist.tile([128, T], BF, tag=f"kT{g}", name=f"kT{g}")
              for g in range(NG)]
        qT = [persist.tile([128, T], BF, tag=f"qT{g}", name=f"qT{g}")
              for g in range(NG)]
        V = persist.tile([128, NTT, HC, 65], BF, tag="V")

        # xT resident in SBUF: one tile PER QUARTER (separate tiles keep
        # the dependency tracking fine-grained, so quarter-0 projections
        # never wait on later quarters' loads).
        xT_q = [persist.tile([128, NCT, 512], BF, tag=f"xTq{rnd}",
                              name=f"xTq{rnd}")
                for rnd in range(NQ)]

        def load_xT(rnd):
            nc.sync.dma_start(
                out=xT_q[rnd],
                in_=x_d.ap()[:, rnd * 512:(rnd + 1) * 512].rearrange(
                    "(ct p) t -> p ct t", p=128))

        load_xT(0)

        # weights: bf16 loads on the scalar HWDGE queue; the sync queue
        # carries xT and the normalization bounce.
        wq_bf = persist.tile([128, NCT, 512], BF, tag="wq_bf")
        wk_bf = persist.tile([128, NCT, 512], BF, tag="wk_bf")
        wv_bf = persist.tile([128, NCT, 512], BF, tag="wv_bf")
        wo_bf = persist.tile([128, NG, C], BF, tag="wo_bf")
        for wdram, wbf in ((wq_d, wq_bf), (wk_d, wk_bf), (wv_d, wv_bf)):
            nc.scalar.dma_start(
                out=wbf, in_=wdram.ap().rearrange("(ct p) m -> p ct m", p=128))
        nc.scalar.dma_start(
            out=wo_bf, in_=wo_d.ap().rearrange("(g p) c -> p g c", p=128))

        # ones column of V (fused softmax denominator)
        ones_f32 = persist.tile([128, NTT, HC], F32, tag="ones")
        nc.vector.memset(ones_f32, 1.0)
        nc.vector.tensor_copy(V[:, :, :, 64], ones_f32)

        # DRAM scratch for the reciprocal broadcast bounce
        rrec_d = [dpool.tile([1, 1024], F32, tag=f"rrec{i}", name=f"rrec{i}",
                             bufs=2)
                  for i in range(NG)]

        def emit_outproj(qb, att):
            # out projection for quarter qb's q rows. Emitted AFTER the
            # next quarter's projections so the shared "pj" PSUM rotation
            # never makes projections wait on the normalization chain.
            for qtl in range(4):
                qt = qb * 4 + qtl
                y_sb = work.tile([128, C], F32, tag="y_sb", bufs=2,
                                 name="y_sb")
                for half in range(2):
                    psy = ps.tile([128, 512], F32, tag="pj", bufs=2,
                                  name="psy")
                    for g in range(NG):
                        nc.tensor.matmul(
                            psy,
                            att[g][:, qtl * 128:(qtl + 1) * 128],
                            wo_bf[:, g, half * 512:(half + 1) * 512],
                            start=(g == 0),
                            stop=(g == NG - 1),
                        )
                    nc.vector.tensor_copy(
                        y_sb[:, half * 512:(half + 1) * 512], psy)
                # y rides the otherwise-idle gpsimd SWDGE ring, except
                # the final quarter which uses the (by then idle) scalar
                # ring for lower tail latency.
                yeng = nc.scalar if qb == 3 else nc.gpsimd
                yeng.dma_start(
                    out=y_d.ap()[qt * 128:(qt + 1) * 128, :], in_=y_sb
                )

        def emit_V(vq):
            # V projection for quarter vq. Must be emitted BEFORE that
            # quarter's attention (program order defines dataflow: the
            # diagonal AVs read it). Quarter vq+1's V is emitted at the
            # END of section vq so a quarter boundary only waits on the
            # next group's q/k chain, never on V.
            for tt in range(4):
                pv = ps.tile([128, HC, 64], F32, tag="pj", bufs=2,
                             name="pv")
                for ct in range(NCT):
                    nc.tensor.matmul(
                        pv,
                        xT_q[vq][:, ct, tt * 128:(tt + 1) * 128],
                        wv_bf[:, ct, :],
                        start=(ct == 0), stop=(ct == NCT - 1),
                    )
                nc.vector.tensor_copy(V[:, vq * 4 + tt, :, 0:64], pv)

        att_q = {}
        for qb in range(NQ):
            q0 = qb * 512
            nkt = 4 * (qb + 1)
            if qb + 1 < NQ:
                load_xT(qb + 1)
            if qb == 0:
                emit_V(0)
            att = [work.tile([128, 512], BF, tag=f"att{g}", name=f"att{g}",
                             bufs=4)
                   for g in range(NG)]
            att_q[qb] = att
            for g in range(NG):
                # ---- qT/kT for group g ----
                for which, wbf, dst in ((0, wq_bf, qT[g]), (1, wk_bf, kT[g])):
                    pj = ps.tile([128, 512], F32, tag="pj", bufs=2, name="pj")
                    for ct in range(NCT):
                        nc.tensor.matmul(
                            pj,
                            wbf[:, ct, g * 128:(g + 1) * 128],
                            xT_q[qb][:, ct, :],
                            start=(ct == 0), stop=(ct == NCT - 1),
                        )
                    nc.vector.tensor_copy(dst[:, q0:q0 + 512], pj)

                # ---- attention for group g ----
                av0 = ps.tile([65, 512], F32, tag="av0", name="av0")
                av1 = ps.tile([65, 512], F32, tag="av1", name="av1")
                for kt in range(nkt):
                    j = kt - 4 * qb
                    # diagonal block: in wT[:, col] (keys on partitions p,
                    # queries on cols) the keep condition is
                    # col - p - 128j >= 0, so cols [0,128j) are fully
                    # masked. Skip them in the scores matmul, the exp and
                    # the AV matmul; only [128j, 128j+128) needs the
                    # triangular select.
                    z = 128 * j if j > 0 else 0
                    sc = ps.tile([128, 1024], F32, tag="sc", bufs=2, name="sc")
                    for hh in range(2):
                        o = hh * 512
                        nc.tensor.matmul(
                            sc[:, o + z:o + 512],
                            kT[g][hh * 64:hh * 64 + 64,
                                  kt * 128:(kt + 1) * 128],
                            qT[g][hh * 64:hh * 64 + 64,
                                  q0 + z:q0 + 512],
                            start=True, stop=True,
                            tile_position=(64 * hh, 0),
                        )
                    wT = work.tile([128, 1024], BF, tag="wT", bufs=4)
                    if j >= 2:
                        # exp only the live columns (2-instruction split
                        # only pays off once >=256 cols are masked)
                        for hh in range(2):
                            o = hh * 512
                            nc.vector.memset(wT[:, o:o + z], 0.0)
                            nc.scalar.activation(
                                wT[:, o + z:o + 512], sc[:, o + z:o + 512],
                                EXP, scale=SCALE)
                            nc.gpsimd.affine_select(
                                out=wT[:, o + z:o + z + 128],
                                in_=wT[:, o + z:o + z + 128],
                                compare_op=mybir.AluOpType.is_ge,
                                fill=0.0,
                                base=0,
                                pattern=[[1, 128]],
                                channel_multiplier=-1,
                            )
                    elif j >= 0:
                        # full-width exp (cols [0,128j) hold stale psum,
                        # exp'd then zeroed by the select below)
                        nc.scalar.activation(wT, sc, EXP, scale=SCALE)
                        ncols = 128 * (j + 1)
                        for hh in range(2):
                            o = hh * 512
                            nc.gpsimd.affine_select(
                                out=wT[:, o:o + ncols],
                                in_=wT[:, o:o + ncols],
                                compare_op=mybir.AluOpType.is_ge,
                                fill=0.0,
                                base=-z,
                                pattern=[[1, ncols]],
                                channel_multiplier=-1,
                            )
                    else:
                        nc.scalar.activation(wT, sc, EXP, scale=SCALE)
                    for hh, av in ((0, av0), (1, av1)):
                        nc.tensor.matmul(
                            av[:, z:512], V[:, kt, 2 * g + hh, :],
                            wT[:, hh * 512 + z:(hh + 1) * 512],
                            start=(kt == 0), stop=(kt == nkt - 1),
                        )

                # ---- normalize: denominators via [128,8] reciprocal ----
                avc = work.tile([65, 1024], F32, tag="avc", bufs=2,
                                name="avc")
                nc.vector.tensor_copy(avc[:, 0:512], av0)
                nc.vector.tensor_copy(avc[:, 512:1024], av1)
                # SBUF->SBUF partition scatter of the denominator row so
                # the reciprocal runs on all 128 DVE lanes.
                # the very last group's chain runs after all exps are done,
                # so it can use the scalar ring and skip the sync-ring
                # backlog of the earlier groups' chains.
                neng = nc.scalar if (qb == 3 and g == 3) else nc.sync
                dsc = work.tile([128, 8], F32, tag="dsc", bufs=2, name="dsc")
                neng.dma_start(out=dsc, in_=avc[64:65, :])
                rec = work.tile([128, 8], F32, tag="rec", bufs=2, name="rec")
                nc.vector.reciprocal(rec, dsc)
                # gather back to DRAM in q-major order per head, then one
                # stride-0 broadcast read for both heads.
                neng.dma_start(
                    out=bass.AP(rrec_d[g].tensor, rrec_d[g].offset,
                                [[8, 128], [1, 8]]),
                    in_=rec)
                rep = work.tile([64, 1024], F32, tag="rep", bufs=2,
                                name="rep")
                neng.dma_start(
                    out=rep,
                    in_=bass.AP(rrec_d[g].tensor, rrec_d[g].offset,
                                [[0, 64], [1, 1024]]))
                nc.vector.tensor_mul(att[g][0:64, :], avc[0:64, 0:512],
                                     rep[:, 0:512])
                tmpB = work.tile([64, 512], BF, tag="tmpB", bufs=2,
                                 name="tmpB")
                nc.vector.tensor_mul(tmpB, avc[0:64, 512:1024],
                                     rep[:, 512:1024])
                neng.dma_start(out=att[g][64:128, :], in_=tmpB)

            if qb + 1 < NQ:
                emit_V(qb + 1)

        # quarters 0-2 out-projections: emitted last (lowest priority) so
        # they act as pure filler for exp-paced attention phases.
        for pqb in range(3):
            emit_outproj(pqb, att_q[pqb])

        # quarter 3's out-projection forms the tail: use the sc PSUM
        # banks (free once the last scores ran) as paired accumulators so
        # two qtl are in flight and each drains with a single copy.
        for qtl in range(4):
            pspair = ps.tile([128, 1024], F32, tag="sc", bufs=2,
                             name="psyt")
            for half in range(2):
                for g in range(NG):
                    nc.tensor.matmul(
                        pspair[:, half * 512:(half + 1) * 512],
                        att_q[3][g][:, qtl * 128:(qtl + 1) * 128],
                        wo_bf[:, g, half * 512:(half + 1) * 512],
                        start=(g == 0), stop=(g == NG - 1),
                    )
            y_sb = work.tile([128, C], F32, tag="y_sb", bufs=2, name="y_sb")
            nc.vector.tensor_copy(y_sb, pspair)
            nc.scalar.dma_start(
                out=y_d.ap()[(12 + qtl) * 128:(13 + qtl) * 128, :], in_=y_sb)


    nc.compile()
    return nc


_NC_CACHE = None


def _get_nc():
    global _NC_CACHE
    if _NC_CACHE is None:
        _NC_CACHE = build_nc()
    return _NC_CACHE


def kernel(x, w_qkv, w_out, _trace=False):
    import ml_dtypes

    bf16 = ml_dtypes.bfloat16
    B = x.shape[0]
    x = np.asarray(x, dtype=np.float32).astype(bf16)
    w_qkv = np.asarray(w_qkv, dtype=np.float32).astype(bf16)
    w_out = np.asarray(w_out, dtype=np.float32).astype(bf16)

    nc = _get_nc()
    in_maps = []
    for core in range(8):
        b = core % B
        hbase = (core // B) * HC
        lo, hi = hbase * D, hbase * D + HC * D
        in_maps.append({
            "x": np.ascontiguousarray(x[b].T),
            "wq": np.ascontiguousarray(w_qkv[:, lo:hi]),
            "wk": np.ascontiguousarray(w_qkv[:, C + lo:C + hi]),
            "wv": np.ascontiguousarray(w_qkv[:, 2 * C + lo:2 * C + hi]),
            "wo": np.ascontiguousarray(w_out[lo:hi, :]),
        })

    res = run_bass_kernel_spmd(nc, in_maps, core_ids=list(range(8)), trace=_trace)
    ys = [r["y"] for r in res.results]
    out = np.empty((B, T, C), dtype=np.float32)
    for b in range(B):
        out[b] = ys[b] + ys[b + B]
    if _trace:
        return out, res
    return out


# revision 51
# speedup vs baseline: 1.0689x; 1.0136x over previous
"""Causal self-attention for trn2, 8 NeuronCores.

Problem: x[4,2048,1024] @ w_qkv[1024,3072] -> causal MHA (16 heads, d=64)
-> @ w_out[1024,1024].

Sharding: core c handles batch b=c%4 and heads hbase=8*(c//4)..hbase+8
(data parallel on B x tensor parallel on heads). Each core computes the
partial out-projection y_c = att_slice @ w_out[slice]; the host sums the
two partials per batch.

v5: restructured from v4 for TensorE saturation.
- All qkv projections run up-front (per quarter), with qT for every
  quarter retained in SBUF. This front-loads TensorE work so the
  scheduler can fill exp-paced attention gaps with projection matmuls.
- Scores for the two heads of a group are row-tiled (K=64 each,
  tile_position (0,0)/(64,0)) and run concurrently into one [128,1024]
  PSUM pair; a single exp covers both heads.
- Softmax denominators come from the fused ones-column in AV (row 64 of
  the [65,512] accumulators). Normalization scatters both denominator
  rows through DRAM into a [128,8] tile so the reciprocal runs on 128
  DVE lanes (~0.1us) instead of one (3.3us), then DMA-broadcasts back.
- Diagonal k-tiles only exp the causal columns (memset the rest).
- PSUM: sc [128,1024]x2 (4 banks) + av0/av1 [65,512] (2 banks) +
  pj [128,512]x2 (2 banks) shared by qkv-proj, V-proj and out-proj.
"""

import sys

for p in ("/opt/trn_rl_repo", "/opt/pypackages"):
    if p not in sys.path:
        sys.path.insert(0, p)

import contextlib

import numpy as np

import concourse.bass as bass
import concourse.mybir as mybir
import concourse.tile as tile
from concourse import bacc
from concourse.bass_utils import run_bass_kernel_spmd
from concourse.masks import make_identity

F32 = mybir.dt.float32
BF = mybir.dt.bfloat16
EXP = mybir.ActivationFunctionType.Exp

T = 2048          # sequence length
C = 1024          # model dim
HC = 8            # heads per core
D = 64            # head dim
NG = 4            # head-groups of 2 per core
NCT = C // 128    # 8 contraction tiles
NTT = T // 128    # 16 token tiles
NQ = 4            # T quarters
SCALE = 0.125     # 1/sqrt(D)


def build_nc():
    nc = bacc.Bacc("TRN2", target_bir_lowering=False, debug=False)

    # All matmul operands are bf16 anyway, so inputs arrive pre-cast to
    # bf16 from the host: halves the startup DMA bytes and removes every
    # staging cast (device cast via DVE would be identical numerics).
    # x additionally arrives PRE-TRANSPOSED ([C, T]) so the xT operand the
    # projections need loads with plain DMAs - no on-chip transposes.
    x_d = nc.dram_tensor("x", [C, T], BF, kind="ExternalInput")
    wq_d = nc.dram_tensor("wq", [C, 512], BF, kind="ExternalInput")
    wk_d = nc.dram_tensor("wk", [C, 512], BF, kind="ExternalInput")
    wv_d = nc.dram_tensor("wv", [C, 512], BF, kind="ExternalInput")
    wo_d = nc.dram_tensor("wo", [512, C], BF, kind="ExternalInput")
    y_d = nc.dram_tensor("y", [T, C], F32, kind="ExternalOutput")

    with tile.TileContext(nc) as tc, contextlib.ExitStack() as ctx:
        persist = ctx.enter_context(tc.tile_pool(name="persist", bufs=1))
        work = ctx.enter_context(tc.tile_pool(name="work", bufs=1))
        ps = ctx.enter_context(tc.tile_pool(name="ps", bufs=1, space="PSUM"))
        dpool = ctx.enter_context(tc.tile_pool(name="dram", bufs=1, space="DRAM"))

        kT = [persist.tile([128, T], BF, tag=f"kT{g}", name=f"kT{g}")
              for g in range(NG)]
        qT = [persist.tile([128, T], BF, tag=f"qT{g}", name=f"qT{g}")
              for g in range(NG)]
        V = persist.tile([128, NTT, HC, 65], BF, tag="V")

        # xT resident in SBUF: one tile PER QUARTER (separate tiles keep
        # the dependency tracking fine-grained, so quarter-0 projections
        # never wait on later quarters' loads).
        xT_q = [persist.tile([128, NCT, 512], BF, tag=f"xTq{rnd}",
                              name=f"xTq{rnd}")
                for rnd in range(NQ)]

        def load_xT(rnd):
            nc.sync.dma_start(
                out=xT_q[rnd],
                in_=x_d.ap()[:, rnd * 512:(rnd + 1) * 512].rearrange(
                    "(ct p) t -> p ct t", p=128))

        for rnd in range(NQ):
            load_xT(rnd)

        # weights: bf16 loads on the scalar HWDGE queue; the sync queue
        # carries xT and the normalization bounce.
        wq_bf = persist.tile([128, NCT, 512], BF, tag="wq_bf")
        wk_bf = persist.tile([128, NCT, 512], BF, tag="wk_bf")
        wv_bf = persist.tile([128, NCT, 512], BF, tag="wv_bf")
        wo_bf = persist.tile([128, NG, C], BF, tag="wo_bf")
        for wdram, wbf in ((wq_d, wq_bf), (wk_d, wk_bf), (wv_d, wv_bf)):
            nc.scalar.dma_start(
                out=wbf, in_=wdram.ap().rearrange("(ct p) m -> p ct m", p=128))
        nc.scalar.dma_start(
            out=wo_bf, in_=wo_d.ap().rearrange("(g p) c -> p g c", p=128))

        # ones column of V (fused softmax denominator)
        ones_f32 = persist.tile([128, NTT, HC], F32, tag="ones")
        nc.vector.memset(ones_f32, 1.0)
        nc.vector.tensor_copy(V[:, :, :, 64], ones_f32)

        # DRAM scratch for the reciprocal broadcast bounce
        rrec_d = [dpool.tile([1, 1024], F32, tag=f"rrec{i}", name=f"rrec{i}",
                             bufs=2)
                  for i in range(NG)]

        def emit_outproj(qb, att):
            # out projection for quarter qb's q rows. Emitted AFTER the
            # next quarter's projections so the shared "pj" PSUM rotation
            # never makes projections wait on the normalization chain.
            for qtl in range(4):
                qt = qb * 4 + qtl
                y_sb = work.tile([128, C], F32, tag="y_sb", bufs=2,
                                 name="y_sb")
                for half in range(2):
                    psy = ps.tile([128, 512], F32, tag="pj", bufs=2,
                                  name="psy")
                    for g in range(NG):
                        nc.tensor.matmul(
                            psy,
                            att[g][:, qtl * 128:(qtl + 1) * 128],
                            wo_bf[:, g, half * 512:(half + 1) * 512],
                            start=(g == 0),
                            stop=(g == NG - 1),
                        )
                    nc.vector.tensor_copy(
                        y_sb[:, half * 512:(half + 1) * 512], psy)
                # y rides the otherwise-idle gpsimd SWDGE ring, except
                # the final quarter which uses the (by then idle) scalar
                # ring for lower tail latency.
                yeng = nc.scalar if qb == 3 else nc.gpsimd
                yeng.dma_start(
                    out=y_d.ap()[qt * 128:(qt + 1) * 128, :], in_=y_sb
                )

        def emit_V(vq):
            # V projection for quarter vq. Must be emitted BEFORE that
            # quarter's attention (program order defines dataflow: the
            # diagonal AVs read it). Quarter vq+1's V is emitted at the
            # END of section vq so a quarter boundary only waits on the
            # next group's q/k chain, never on V.
            for tt in range(4):
                pv = ps.tile([128, HC, 64], F32, tag="pj", bufs=2,
                             name="pv")
                for ct in range(NCT):
                    nc.tensor.matmul(
                        pv,
                        xT_q[vq][:, ct, tt * 128:(tt + 1) * 128],
                        wv_bf[:, ct, :],
                        start=(ct == 0), stop=(ct == NCT - 1),
                    )
                nc.vector.tensor_copy(V[:, vq * 4 + tt, :, 0:64], pv)

        att_q = {}
        for qb in range(NQ):
            q0 = qb * 512
            nkt = 4 * (qb + 1)
            att = [work.tile([128, 512], BF, tag=f"att{g}", name=f"att{g}",
                             bufs=4)
                   for g in range(NG)]
            att_q[qb] = att
            for g in range(NG):
                # ---- qT/kT for group g ----
                for which, wbf, dst in ((0, wq_bf, qT[g]), (1, wk_bf, kT[g])):
                    pj = ps.tile([128, 512], F32, tag="pj", bufs=2, name="pj")
                    for ct in range(NCT):
                        nc.tensor.matmul(
                            pj,
                            wbf[:, ct, g * 128:(g + 1) * 128],
                            xT_q[qb][:, ct, :],
                            start=(ct == 0), stop=(ct == NCT - 1),
                        )
                    nc.vector.tensor_copy(dst[:, q0:q0 + 512], pj)

                if g == 0:
                    emit_V(qb)

                # ---- attention for group g ----
                av0 = ps.tile([65, 512], F32, tag="av0", name="av0")
                av1 = ps.tile([65, 512], F32, tag="av1", name="av1")
                for kt in range(nkt):
                    j = kt - 4 * qb
                    # diagonal block: in wT[:, col] (keys on partitions p,
                    # queries on cols) the keep condition is
                    # col - p - 128j >= 0, so cols [0,128j) are fully
                    # masked. Skip them in the scores matmul, the exp and
                    # the AV matmul; only [128j, 128j+128) needs the
                    # triangular select.
                    z = 128 * j if j > 0 else 0
                    sc = ps.tile([128, 1024], F32, tag="sc", bufs=2, name="sc")
                    for hh in range(2):
                        o = hh * 512
                        nc.tensor.matmul(
                            sc[:, o + z:o + 512],
                            kT[g][hh * 64:hh * 64 + 64,
                                  kt * 128:(kt + 1) * 128],
                            qT[g][hh * 64:hh * 64 + 64,
                                  q0 + z:q0 + 512],
                            start=True, stop=True,
                            tile_position=(64 * hh, 0),
                        )
                    wT = work.tile([128, 1024], BF, tag="wT", bufs=4)
                    if j >= 2:
                        # exp only the live columns (2-instruction split
                        # only pays off once >=256 cols are masked)
                        for hh in range(2):
                            o = hh * 512
                            nc.vector.memset(wT[:, o:o + z], 0.0)
                            nc.scalar.activation(
                                wT[:, o + z:o + 512], sc[:, o + z:o + 512],
                                EXP, scale=SCALE)
                            nc.gpsimd.affine_select(
                                out=wT[:, o + z:o + z + 128],
                                in_=wT[:, o + z:o + z + 128],
                                compare_op=mybir.AluOpType.is_ge,
                                fill=0.0,
                                base=0,
                                pattern=[[1, 128]],
                                channel_multiplier=-1,
                            )
                    elif j >= 0:
                        # full-width exp (cols [0,128j) hold stale psum,
                        # exp'd then zeroed by the select below)
                        nc.scalar.activation(wT, sc, EXP, scale=SCALE)
                        ncols = 128 * (j + 1)
                        for hh in range(2):
                            o = hh * 512
                            nc.gpsimd.affine_select(
                                out=wT[:, o:o + ncols],
                                in_=wT[:, o:o + ncols],
                                compare_op=mybir.AluOpType.is_ge,
                                fill=0.0,
                                base=-z,
                                pattern=[[1, ncols]],
                                channel_multiplier=-1,
                            )
                    else:
                        nc.scalar.activation(wT, sc, EXP, scale=SCALE)
                    for hh, av in ((0, av0), (1, av1)):
                        nc.tensor.matmul(
                            av[:, z:512], V[:, kt, 2 * g + hh, :],
                            wT[:, hh * 512 + z:(hh + 1) * 512],
                            start=(kt == 0), stop=(kt == nkt - 1),
                        )

                # ---- normalize: denominators via [128,8] reciprocal ----
                avc = work.tile([65, 1024], F32, tag="avc", bufs=2,
                                name="avc")
                nc.vector.tensor_copy(avc[:, 0:512], av0)
                nc.vector.tensor_copy(avc[:, 512:1024], av1)
                # SBUF->SBUF partition scatter of the denominator row so
                # the reciprocal runs on all 128 DVE lanes.
                # the very last group's chain runs after all exps are done,
                # so it can use the scalar ring and skip the sync-ring
                # backlog of the earlier groups' chains.
                neng = nc.scalar if (qb == 3 and g == 3) else nc.sync
                dsc = work.tile([128, 8], F32, tag="dsc", bufs=2, name="dsc")
                neng.dma_start(out=dsc, in_=avc[64:65, :])
                rec = work.tile([128, 8], F32, tag="rec", bufs=2, name="rec")
                nc.vector.reciprocal(rec, dsc)
                # gather back to DRAM in q-major order per head, then one
                # stride-0 broadcast read for both heads.
                neng.dma_start(
                    out=bass.AP(rrec_d[g].tensor, rrec_d[g].offset,
                                [[8, 128], [1, 8]]),
                    in_=rec)
                rep = work.tile([64, 1024], F32, tag="rep", bufs=2,
                                name="rep")
                neng.dma_start(
                    out=rep,
                    in_=bass.AP(rrec_d[g].tensor, rrec_d[g].offset,
                                [[0, 64], [1, 1024]]))
                nc.vector.tensor_mul(att[g][0:64, :], avc[0:64, 0:512],
                                     rep[:, 0:512])
                tmpB = work.tile([64, 512], BF, tag="tmpB", bufs=2,
                                 name="tmpB")
                nc.vector.tensor_mul(tmpB, avc[0:64, 512:1024],
                                     rep[:, 512:1024])
                neng.dma_start(out=att[g][64:128, :], in_=tmpB)

        # quarters 0-2 out-projections: emitted last (lowest priority) so
        # they act as pure filler for exp-paced attention phases.
        for pqb in range(3):
            emit_outproj(pqb, att_q[pqb])

        # quarter 3's out-projection forms the tail: use the sc PSUM
        # banks (free once the last scores ran) as paired accumulators so
        # two qtl are in flight and each drains with a single copy.
        for qtl in range(4):
            pspair = ps.tile([128, 1024], F32, tag="sc", bufs=2,
                             name="psyt")
            for half in range(2):
                for g in range(NG):
                    nc.tensor.matmul(
                        pspair[:, half * 512:(half + 1) * 512],
                        att_q[3][g][:, qtl * 128:(qtl + 1) * 128],
                        wo_bf[:, g, half * 512:(half + 1) * 512],
                        start=(g == 0), stop=(g == NG - 1),
                    )
            y_sb = work.tile([128, C], F32, tag="y_sb", bufs=2, name="y_sb")
            nc.vector.tensor_copy(y_sb, pspair)
            nc.scalar.dma_start(
                out=y_d.ap()[(12 + qtl) * 128:(13 + qtl) * 128, :], in_=y_sb)


    nc.compile()
    return nc


_NC_CACHE = None


def _get_nc():
    global _NC_CACHE
    if _NC_CACHE is None:
        _NC_CACHE = build_nc()
    return _NC_CACHE


def kernel(x, w_qkv, w_out, _trace=False):
    import ml_dtypes

    bf16 = ml_dtypes.bfloat16
    B = x.shape[0]
    x = np.asarray(x, dtype=np.float32).astype(bf16)
    w_qkv = np.asarray(w_qkv, dtype=np.float32).astype(bf16)
    w_out = np.asarray(w_out, dtype=np.float32).astype(bf16)

    nc = _get_nc()
    in_maps = []
    for core in range(8):
        b = core % B
        hbase = (core // B) * HC
        lo, hi = hbase * D, hbase * D + HC * D
        in_maps.append({
            "x": np.ascontiguousarray(x[b].T),
            "wq": np.ascontiguousarray(w_qkv[:, lo:hi]),
            "wk": np.ascontiguousarray(w_qkv[:, C + lo:C + hi]),
            "wv": np.ascontiguousarray(w_qkv[:, 2 * C + lo:2 * C + hi]),
            "wo": np.ascontiguousarray(w_out[lo:hi, :]),
        })

    res = run_bass_kernel_spmd(nc, in_maps, core_ids=list(range(8)), trace=_trace)
    ys = [r["y"] for r in res.results]
    out = np.empty((B, T, C), dtype=np.float32)
    for b in range(B):
        out[b] = ys[b] + ys[b + B]
    if _trace:
        return out, res
    return out


# revision 52
# speedup vs baseline: 1.0762x; 1.0068x over previous
"""Causal self-attention for trn2, 8 NeuronCores.

Problem: x[4,2048,1024] @ w_qkv[1024,3072] -> causal MHA (16 heads, d=64)
-> @ w_out[1024,1024].

Sharding: core c handles batch b=c%4 and heads hbase=8*(c//4)..hbase+8
(data parallel on B x tensor parallel on heads). Each core computes the
partial out-projection y_c = att_slice @ w_out[slice]; the host sums the
two partials per batch.

Final design (v18, ~295us vs 519us baseline):
- Inputs arrive host-side pre-cast to bf16, with x pre-TRANSPOSED
  ([C,T]) so xT loads as plain DMAs into persistent SBUF tiles - no
  on-chip transposes at all.
- Per quarter, per head-group: q/k projection chains, then that group's
  attention immediately (early exps); V projection emitted before the
  first group's attention (program order defines dataflow for the
  diagonal AV reads).
- Scores for the two heads of a group are row-tiled (K=64 each,
  tile_position (0,0)/(64,0)) and run CONCURRENTLY into one [128,1024]
  PSUM pair; a single ScalarE exp covers both heads. The kernel is
  exp-paced (~1.12us per k-tile); projection/out-projection matmuls
  fill the TensorE gaps via the Tile scheduler.
- Softmax denominators come from the fused ones-column in AV (row 64 of
  the [65,512] accumulators). Normalization scatters the denominator
  row into a [128,8] tile so the reciprocal runs on all 128 DVE lanes,
  then DMA-broadcasts the reciprocals back via a DRAM bounce.
- Diagonal k-tiles: fully-masked leading columns are skipped in the
  scores matmul, exp and AV (N-trim); only a 128-wide band needs the
  gpsimd triangular select.
- DMA rings: sync = xT + normalization chains (engine-blocking waits are
  free there); scalar = weights + tail-quarter y (doorbells cost ~0.6us
  of issuing-engine time -> never put dep-waiting DMAs on scalar, which
  paces the exps); gpsimd SWDGE = bulk y writes.
- Out-projections for quarters 0-2 are emitted last (lowest priority,
  pure filler); quarter 3's uses the freed sc PSUM banks as paired
  accumulators to shorten the tail.
- PSUM: sc [128,1024]x2 (4 banks) + av0/av1 [65,512] (2 banks) +
  pj [128,512]x2 (2 banks) shared by qkv-proj, V-proj and out-proj.
"""

import sys

for p in ("/opt/trn_rl_repo", "/opt/pypackages"):
    if p not in sys.path:
        sys.path.insert(0, p)

import contextlib

import numpy as np

import concourse.bass as bass
import concourse.mybir as mybir
import concourse.tile as tile
from concourse import bacc
from concourse.bass_utils import run_bass_kernel_spmd
from concourse.masks import make_identity

F32 = mybir.dt.float32
BF = mybir.dt.bfloat16
EXP = mybir.ActivationFunctionType.Exp

T = 2048          # sequence length
C = 1024          # model dim
HC = 8            # heads per core
D = 64            # head dim
NG = 4            # head-groups of 2 per core
NCT = C // 128    # 8 contraction tiles
NTT = T // 128    # 16 token tiles
NQ = 4            # T quarters
SCALE = 0.125     # 1/sqrt(D)


def build_nc():
    nc = bacc.Bacc("TRN2", target_bir_lowering=False, debug=False)

    # All matmul operands are bf16 anyway, so inputs arrive pre-cast to
    # bf16 from the host: halves the startup DMA bytes and removes every
    # staging cast (device cast via DVE would be identical numerics).
    # x additionally arrives PRE-TRANSPOSED ([C, T]) so the xT operand the
    # projections need loads with plain DMAs - no on-chip transposes.
    x_d = nc.dram_tensor("x", [C, T], BF, kind="ExternalInput")
    wq_d = nc.dram_tensor("wq", [C, 512], BF, kind="ExternalInput")
    wk_d = nc.dram_tensor("wk", [C, 512], BF, kind="ExternalInput")
    wv_d = nc.dram_tensor("wv", [C, 512], BF, kind="ExternalInput")
    wo_d = nc.dram_tensor("wo", [512, C], BF, kind="ExternalInput")
    y_d = nc.dram_tensor("y", [T, C], F32, kind="ExternalOutput")

    with tile.TileContext(nc) as tc, contextlib.ExitStack() as ctx:
        persist = ctx.enter_context(tc.tile_pool(name="persist", bufs=1))
        work = ctx.enter_context(tc.tile_pool(name="work", bufs=1))
        ps = ctx.enter_context(tc.tile_pool(name="ps", bufs=1, space="PSUM"))
        dpool = ctx.enter_context(tc.tile_pool(name="dram", bufs=1, space="DRAM"))

        kT = [persist.tile([128, T], BF, tag=f"kT{g}", name=f"kT{g}")
              for g in range(NG)]
        qT = [persist.tile([128, T], BF, tag=f"qT{g}", name=f"qT{g}")
              for g in range(NG)]
        V = persist.tile([128, NTT, HC, 65], BF, tag="V")

        # xT resident in SBUF: one tile PER QUARTER (separate tiles keep
        # the dependency tracking fine-grained, so quarter-0 projections
        # never wait on later quarters' loads).
        xT_q = [persist.tile([128, NCT, 512], BF, tag=f"xTq{rnd}",
                              name=f"xTq{rnd}")
                for rnd in range(NQ)]

        def load_xT(rnd):
            nc.sync.dma_start(
                out=xT_q[rnd],
                in_=x_d.ap()[:, rnd * 512:(rnd + 1) * 512].rearrange(
                    "(ct p) t -> p ct t", p=128))

        for rnd in range(NQ):
            load_xT(rnd)

        # weights: bf16 loads on the scalar HWDGE queue; the sync queue
        # carries xT and the normalization bounce.
        wq_bf = persist.tile([128, NCT, 512], BF, tag="wq_bf")
        wk_bf = persist.tile([128, NCT, 512], BF, tag="wk_bf")
        wv_bf = persist.tile([128, NCT, 512], BF, tag="wv_bf")
        wo_bf = persist.tile([128, NG, C], BF, tag="wo_bf")
        for wdram, wbf in ((wq_d, wq_bf), (wk_d, wk_bf), (wv_d, wv_bf)):
            nc.scalar.dma_start(
                out=wbf, in_=wdram.ap().rearrange("(ct p) m -> p ct m", p=128))
        nc.scalar.dma_start(
            out=wo_bf, in_=wo_d.ap().rearrange("(g p) c -> p g c", p=128))

        # ones column of V (fused softmax denominator)
        ones_f32 = persist.tile([128, NTT, HC], F32, tag="ones")
        nc.vector.memset(ones_f32, 1.0)
        nc.vector.tensor_copy(V[:, :, :, 64], ones_f32)

        # DRAM scratch for the reciprocal broadcast bounce
        rrec_d = [dpool.tile([1, 1024], F32, tag=f"rrec{i}", name=f"rrec{i}",
                             bufs=2)
                  for i in range(NG)]

        def emit_outproj(qb, att):
            # out projection for quarter qb's q rows. Emitted AFTER the
            # next quarter's projections so the shared "pj" PSUM rotation
            # never makes projections wait on the normalization chain.
            for qtl in range(4):
                qt = qb * 4 + qtl
                y_sb = work.tile([128, C], F32, tag="y_sb", bufs=2,
                                 name="y_sb")
                for half in range(2):
                    psy = ps.tile([128, 512], F32, tag="pj", bufs=2,
                                  name="psy")
                    for g in range(NG):
                        nc.tensor.matmul(
                            psy,
                            att[g][:, qtl * 128:(qtl + 1) * 128],
                            wo_bf[:, g, half * 512:(half + 1) * 512],
                            start=(g == 0),
                            stop=(g == NG - 1),
                        )
                    nc.vector.tensor_copy(
                        y_sb[:, half * 512:(half + 1) * 512], psy)
                # y rides the otherwise-idle gpsimd SWDGE ring, except
                # the final quarter which uses the (by then idle) scalar
                # ring for lower tail latency.
                yeng = nc.scalar if qb == 3 else nc.gpsimd
                yeng.dma_start(
                    out=y_d.ap()[qt * 128:(qt + 1) * 128, :], in_=y_sb
                )

        def emit_V(vq):
            # V projection for quarter vq. Must be emitted BEFORE that
            # quarter's attention (program order defines dataflow: the
            # diagonal AVs read it). Quarter vq+1's V is emitted at the
            # END of section vq so a quarter boundary only waits on the
            # next group's q/k chain, never on V.
            for tt in range(4):
                pv = ps.tile([128, HC, 64], F32, tag="pj", bufs=2,
                             name="pv")
                for ct in range(NCT):
                    nc.tensor.matmul(
                        pv,
                        xT_q[vq][:, ct, tt * 128:(tt + 1) * 128],
                        wv_bf[:, ct, :],
                        start=(ct == 0), stop=(ct == NCT - 1),
                    )
                nc.vector.tensor_copy(V[:, vq * 4 + tt, :, 0:64], pv)

        att_q = {}
        for qb in range(NQ):
            q0 = qb * 512
            nkt = 4 * (qb + 1)
            att = [work.tile([128, 512], BF, tag=f"att{g}", name=f"att{g}",
                             bufs=4)
                   for g in range(NG)]
            att_q[qb] = att
            for g in range(NG):
                # ---- qT/kT for group g ----
                for which, wbf, dst in ((0, wq_bf, qT[g]), (1, wk_bf, kT[g])):
                    pj = ps.tile([128, 512], F32, tag="pj", bufs=2, name="pj")
                    for ct in range(NCT):
                        nc.tensor.matmul(
                            pj,
                            wbf[:, ct, g * 128:(g + 1) * 128],
                            xT_q[qb][:, ct, :],
                            start=(ct == 0), stop=(ct == NCT - 1),
                        )
                    nc.vector.tensor_copy(dst[:, q0:q0 + 512], pj)

                if g == 0:
                    emit_V(qb)

                # ---- attention for group g ----
                av0 = ps.tile([65, 512], F32, tag="av0", name="av0")
                av1 = ps.tile([65, 512], F32, tag="av1", name="av1")
                for kt in range(nkt):
                    j = kt - 4 * qb
                    # diagonal block: in wT[:, col] (keys on partitions p,
                    # queries on cols) the keep condition is
                    # col - p - 128j >= 0, so cols [0,128j) are fully
                    # masked. Skip them in the scores matmul, the exp and
                    # the AV matmul; only [128j, 128j+128) needs the
                    # triangular select.
                    z = 128 * j if j > 0 else 0
                    sc = ps.tile([128, 1024], F32, tag="sc", bufs=2, name="sc")
                    for hh in range(2):
                        o = hh * 512
                        nc.tensor.matmul(
                            sc[:, o + z:o + 512],
                            kT[g][hh * 64:hh * 64 + 64,
                                  kt * 128:(kt + 1) * 128],
                            qT[g][hh * 64:hh * 64 + 64,
                                  q0 + z:q0 + 512],
                            start=True, stop=True,
                            tile_position=(64 * hh, 0),
                        )
                    wT = work.tile([128, 1024], BF, tag="wT", bufs=4)
                    if j >= 2:
                        # exp only the live columns (2-instruction split
                        # only pays off once >=256 cols are masked)
                        for hh in range(2):
                            o = hh * 512
                            nc.vector.memset(wT[:, o:o + z], 0.0)
                            nc.scalar.activation(
                                wT[:, o + z:o + 512], sc[:, o + z:o + 512],
                                EXP, scale=SCALE)
                            nc.gpsimd.affine_select(
                                out=wT[:, o + z:o + z + 128],
                                in_=wT[:, o + z:o + z + 128],
                                compare_op=mybir.AluOpType.is_ge,
                                fill=0.0,
                                base=0,
                                pattern=[[1, 128]],
                                channel_multiplier=-1,
                            )
                    elif j >= 0:
                        # full-width exp (cols [0,128j) hold stale psum,
                        # exp'd then zeroed by the select below)
                        nc.scalar.activation(wT, sc, EXP, scale=SCALE)
                        ncols = 128 * (j + 1)
                        for hh in range(2):
                            o = hh * 512
                            nc.gpsimd.affine_select(
                                out=wT[:, o:o + ncols],
                                in_=wT[:, o:o + ncols],
                                compare_op=mybir.AluOpType.is_ge,
                                fill=0.0,
                                base=-z,
                                pattern=[[1, ncols]],
                                channel_multiplier=-1,
                            )
                    else:
                        nc.scalar.activation(wT, sc, EXP, scale=SCALE)
                    for hh, av in ((0, av0), (1, av1)):
                        nc.tensor.matmul(
                            av[:, z:512], V[:, kt, 2 * g + hh, :],
                            wT[:, hh * 512 + z:(hh + 1) * 512],
                            start=(kt == 0), stop=(kt == nkt - 1),
                        )

                # ---- normalize: denominators via [128,8] reciprocal ----
                avc = work.tile([65, 1024], F32, tag="avc", bufs=2,
                                name="avc")
                nc.vector.tensor_copy(avc[:, 0:512], av0)
                nc.vector.tensor_copy(avc[:, 512:1024], av1)
                # SBUF->SBUF partition scatter of the denominator row so
                # the reciprocal runs on all 128 DVE lanes.
                # the very last group's chain runs after all exps are done,
                # so it can use the scalar ring and skip the sync-ring
                # backlog of the earlier groups' chains.
                neng = nc.scalar if (qb == 3 and g == 3) else nc.sync
                dsc = work.tile([128, 8], F32, tag="dsc", bufs=2, name="dsc")
                neng.dma_start(out=dsc, in_=avc[64:65, :])
                rec = work.tile([128, 8], F32, tag="rec", bufs=2, name="rec")
                nc.vector.reciprocal(rec, dsc)
                # gather back to DRAM in q-major order per head, then one
                # stride-0 broadcast read for both heads.
                neng.dma_start(
                    out=bass.AP(rrec_d[g].tensor, rrec_d[g].offset,
                                [[8, 128], [1, 8]]),
                    in_=rec)
                rep = work.tile([64, 1024], F32, tag="rep", bufs=2,
                                name="rep")
                neng.dma_start(
                    out=rep,
                    in_=bass.AP(rrec_d[g].tensor, rrec_d[g].offset,
                                [[0, 64], [1, 1024]]))
                nc.vector.tensor_mul(att[g][0:64, :], avc[0:64, 0:512],
                                     rep[:, 0:512])
                tmpB = work.tile([64, 512], BF, tag="tmpB", bufs=2,
                                 name="tmpB")
                nc.vector.tensor_mul(tmpB, avc[0:64, 512:1024],
                                     rep[:, 512:1024])
                neng.dma_start(out=att[g][64:128, :], in_=tmpB)

        # quarters 0-2 out-projections: emitted last (lowest priority) so
        # they act as pure filler for exp-paced attention phases.
        for pqb in range(3):
            emit_outproj(pqb, att_q[pqb])

        # quarter 3's out-projection forms the tail: use the sc PSUM
        # banks (free once the last scores ran) as paired accumulators so
        # two qtl are in flight and each drains with a single copy.
        for qtl in range(4):
            pspair = ps.tile([128, 1024], F32, tag="sc", bufs=2,
                             name="psyt")
            for half in range(2):
                for g in range(NG):
                    nc.tensor.matmul(
                        pspair[:, half * 512:(half + 1) * 512],
                        att_q[3][g][:, qtl * 128:(qtl + 1) * 128],
                        wo_bf[:, g, half * 512:(half + 1) * 512],
                        start=(g == 0), stop=(g == NG - 1),
                    )
            y_sb = work.tile([128, C], F32, tag="y_sb", bufs=2, name="y_sb")
            nc.vector.tensor_copy(y_sb, pspair)
            nc.scalar.dma_start(
                out=y_d.ap()[(12 + qtl) * 128:(13 + qtl) * 128, :], in_=y_sb)


    nc.compile()
    return nc


_NC_CACHE = None


def _get_nc():
    global _NC_CACHE
    if _NC_CACHE is None:
        _NC_CACHE = build_nc()
    return _NC_CACHE


def kernel(x, w_qkv, w_out, _trace=False):
    import ml_dtypes

    bf16 = ml_dtypes.bfloat16
    B = x.shape[0]
    x = np.asarray(x, dtype=np.float32).astype(bf16)
    w_qkv = np.asarray(w_qkv, dtype=np.float32).astype(bf16)
    w_out = np.asarray(w_out, dtype=np.float32).astype(bf16)

    nc = _get_nc()
    in_maps = []
    for core in range(8):
        b = core % B
        hbase = (core // B) * HC
        lo, hi = hbase * D, hbase * D + HC * D
        in_maps.append({
            "x": np.ascontiguousarray(x[b].T),
            "wq": np.ascontiguousarray(w_qkv[:, lo:hi]),
            "wk": np.ascontiguousarray(w_qkv[:, C + lo:C + hi]),
            "wv": np.ascontiguousarray(w_qkv[:, 2 * C + lo:2 * C + hi]),
            "wo": np.ascontiguousarray(w_out[lo:hi, :]),
        })

    res = run_bass_kernel_spmd(nc, in_maps, core_ids=list(range(8)), trace=_trace)
    ys = [r["y"] for r in res.results]
    out = np.empty((B, T, C), dtype=np.float32)
    for b in range(B):
        out[b] = ys[b] + ys[b + B]
    if _trace:
        return out, res
    return out


# revision 53
# speedup vs baseline: 1.0910x; 1.0138x over previous
"""Causal self-attention for trn2, 8 NeuronCores.

Problem: x[4,2048,1024] @ w_qkv[1024,3072] -> causal MHA (16 heads, d=64)
-> @ w_out[1024,1024].

Sharding: core c handles batch b=c%4 and heads hbase=8*(c//4)..hbase+8
(data parallel on B x tensor parallel on heads). Each core computes the
partial out-projection y_c = att_slice @ w_out[slice]; the host sums the
two partials per batch.

Final design (v18, ~295us vs 519us baseline):
- Inputs arrive host-side pre-cast to bf16, with x pre-TRANSPOSED
  ([C,T]) so xT loads as plain DMAs into persistent SBUF tiles - no
  on-chip transposes at all.
- Per quarter, per head-group: q/k projection chains, then that group's
  attention immediately (early exps); V projection emitted before the
  first group's attention (program order defines dataflow for the
  diagonal AV reads).
- Scores for the two heads of a group are row-tiled (K=64 each,
  tile_position (0,0)/(64,0)) and run CONCURRENTLY into one [128,1024]
  PSUM pair; a single ScalarE exp covers both heads. The kernel is
  exp-paced (~1.12us per k-tile); projection/out-projection matmuls
  fill the TensorE gaps via the Tile scheduler.
- Softmax denominators come from the fused ones-column in AV (row 64 of
  the [65,512] accumulators). Normalization scatters the denominator
  row into a [128,8] tile so the reciprocal runs on all 128 DVE lanes,
  then DMA-broadcasts the reciprocals back via a DRAM bounce.
- Diagonal k-tiles: fully-masked leading columns are skipped in the
  scores matmul, exp and AV (N-trim); only a 128-wide band needs the
  gpsimd triangular select.
- DMA rings: sync = xT + normalization chains (engine-blocking waits are
  free there); scalar = weights + tail-quarter y (doorbells cost ~0.6us
  of issuing-engine time -> never put dep-waiting DMAs on scalar, which
  paces the exps); gpsimd SWDGE = bulk y writes.
- Out-projections for quarters 0-2 are emitted last (lowest priority,
  pure filler); quarter 3's uses the freed sc PSUM banks as paired
  accumulators to shorten the tail.
- PSUM: sc [128,1024]x2 (4 banks) + av0/av1 [65,512] (2 banks) +
  pj [128,512]x2 (2 banks) shared by qkv-proj, V-proj and out-proj.
"""

import sys

for p in ("/opt/trn_rl_repo", "/opt/pypackages"):
    if p not in sys.path:
        sys.path.insert(0, p)

import contextlib

import numpy as np

import concourse.bass as bass
import concourse.mybir as mybir
import concourse.tile as tile
from concourse import bacc
from concourse.bass_utils import run_bass_kernel_spmd
from concourse.masks import make_identity

F32 = mybir.dt.float32
BF = mybir.dt.bfloat16
EXP = mybir.ActivationFunctionType.Exp

T = 2048          # sequence length
C = 1024          # model dim
HC = 8            # heads per core
D = 64            # head dim
NG = 4            # head-groups of 2 per core
NCT = C // 128    # 8 contraction tiles
NTT = T // 128    # 16 token tiles
NQ = 4            # T quarters
SCALE = 0.125     # 1/sqrt(D)


def build_nc():
    nc = bacc.Bacc("TRN2", target_bir_lowering=False, debug=False)

    # All matmul operands are bf16 anyway, so inputs arrive pre-cast to
    # bf16 from the host: halves the startup DMA bytes and removes every
    # staging cast (device cast via DVE would be identical numerics).
    # x additionally arrives PRE-TRANSPOSED ([C, T]) so the xT operand the
    # projections need loads with plain DMAs - no on-chip transposes.
    x_d = nc.dram_tensor("x", [C, T], BF, kind="ExternalInput")
    wq_d = nc.dram_tensor("wq", [C, 512], BF, kind="ExternalInput")
    wk_d = nc.dram_tensor("wk", [C, 512], BF, kind="ExternalInput")
    wv_d = nc.dram_tensor("wv", [C, 512], BF, kind="ExternalInput")
    wo_d = nc.dram_tensor("wo", [512, C], BF, kind="ExternalInput")
    y_d = nc.dram_tensor("y", [T, C], F32, kind="ExternalOutput")

    with tile.TileContext(nc) as tc, contextlib.ExitStack() as ctx:
        persist = ctx.enter_context(tc.tile_pool(name="persist", bufs=1))
        work = ctx.enter_context(tc.tile_pool(name="work", bufs=1))
        ps = ctx.enter_context(tc.tile_pool(name="ps", bufs=1, space="PSUM"))
        dpool = ctx.enter_context(tc.tile_pool(name="dram", bufs=1, space="DRAM"))

        kT = [persist.tile([128, T], BF, tag=f"kT{g}", name=f"kT{g}")
              for g in range(NG)]
        qT = [persist.tile([128, T], BF, tag=f"qT{g}", name=f"qT{g}")
              for g in range(NG)]
        V = persist.tile([128, NTT, HC, 65], BF, tag="V")

        # xT resident in SBUF: one tile PER QUARTER (separate tiles keep
        # the dependency tracking fine-grained, so quarter-0 projections
        # never wait on later quarters' loads).
        xT_q = [persist.tile([128, NCT, 512], BF, tag=f"xTq{rnd}",
                              name=f"xTq{rnd}")
                for rnd in range(NQ)]

        def load_xT(rnd):
            nc.sync.dma_start(
                out=xT_q[rnd],
                in_=x_d.ap()[:, rnd * 512:(rnd + 1) * 512].rearrange(
                    "(ct p) t -> p ct t", p=128))

        for rnd in range(NQ):
            load_xT(rnd)

        # weights: bf16 loads on the scalar HWDGE queue; the sync queue
        # carries xT and the normalization bounce.
        wq_bf = persist.tile([128, NCT, 512], BF, tag="wq_bf")
        wk_bf = persist.tile([128, NCT, 512], BF, tag="wk_bf")
        wv_bf = persist.tile([128, NCT, 512], BF, tag="wv_bf")
        wo_bf = persist.tile([128, NG, C], BF, tag="wo_bf")
        for wdram, wbf in ((wq_d, wq_bf), (wk_d, wk_bf), (wv_d, wv_bf)):
            nc.scalar.dma_start(
                out=wbf, in_=wdram.ap().rearrange("(ct p) m -> p ct m", p=128))
        nc.scalar.dma_start(
            out=wo_bf, in_=wo_d.ap().rearrange("(g p) c -> p g c", p=128))

        # ones column of V (fused softmax denominator)
        ones_f32 = persist.tile([128, NTT, HC], F32, tag="ones")
        nc.vector.memset(ones_f32, 1.0)
        nc.vector.tensor_copy(V[:, :, :, 64], ones_f32)

        # DRAM scratch for the reciprocal broadcast bounce
        rrec_d = [dpool.tile([1, 1024], F32, tag=f"rrec{i}", name=f"rrec{i}",
                             bufs=2)
                  for i in range(NG)]

        def emit_outproj(qb, att):
            # out projection for quarter qb's q rows. Emitted AFTER the
            # next quarter's projections so the shared "pj" PSUM rotation
            # never makes projections wait on the normalization chain.
            for qtl in range(4):
                qt = qb * 4 + qtl
                y_sb = work.tile([128, C], F32, tag="y_sb", bufs=2,
                                 name="y_sb")
                for half in range(2):
                    psy = ps.tile([128, 512], F32, tag="pj", bufs=2,
                                  name="psy")
                    for g in range(NG):
                        nc.tensor.matmul(
                            psy,
                            att[g][:, qtl * 128:(qtl + 1) * 128],
                            wo_bf[:, g, half * 512:(half + 1) * 512],
                            start=(g == 0),
                            stop=(g == NG - 1),
                        )
                    nc.vector.tensor_copy(
                        y_sb[:, half * 512:(half + 1) * 512], psy)
                # y rides the otherwise-idle gpsimd SWDGE ring, except
                # the final quarter which uses the (by then idle) scalar
                # ring for lower tail latency.
                yeng = nc.scalar if qb == 3 else nc.gpsimd
                yeng.dma_start(
                    out=y_d.ap()[qt * 128:(qt + 1) * 128, :], in_=y_sb
                )

        def emit_V(vq):
            # V projection for quarter vq. Must be emitted BEFORE that
            # quarter's attention (program order defines dataflow: the
            # diagonal AVs read it). Quarter vq+1's V is emitted at the
            # END of section vq so a quarter boundary only waits on the
            # next group's q/k chain, never on V.
            for tt in range(4):
                pv = ps.tile([128, HC, 64], F32, tag="pj", bufs=2,
                             name="pv")
                for ct in range(NCT):
                    nc.tensor.matmul(
                        pv,
                        xT_q[vq][:, ct, tt * 128:(tt + 1) * 128],
                        wv_bf[:, ct, :],
                        start=(ct == 0), stop=(ct == NCT - 1),
                    )
                nc.vector.tensor_copy(V[:, vq * 4 + tt, :, 0:64], pv)

        att_q = {}
        for qb in range(NQ):
            q0 = qb * 512
            nkt = 4 * (qb + 1)
            att = [work.tile([128, 512], BF, tag=f"att{g}", name=f"att{g}",
                             bufs=4)
                   for g in range(NG)]
            att_q[qb] = att
            for g in range(NG):
                # ---- qT/kT for group g ----
                for which, wbf, dst in ((0, wq_bf, qT[g]), (1, wk_bf, kT[g])):
                    pj = ps.tile([128, 512], F32, tag="pj", bufs=2, name="pj")
                    for ct in range(NCT):
                        nc.tensor.matmul(
                            pj,
                            wbf[:, ct, g * 128:(g + 1) * 128],
                            xT_q[qb][:, ct, :],
                            start=(ct == 0), stop=(ct == NCT - 1),
                        )
                    nc.vector.tensor_copy(dst[:, q0:q0 + 512], pj)

                if g == 0:
                    emit_V(qb)

                # ---- attention for group g ----
                av0 = ps.tile([65, 512], F32, tag="av0", name="av0")
                av1 = ps.tile([65, 512], F32, tag="av1", name="av1")
                for kt in range(nkt):
                    j = kt - 4 * qb
                    # diagonal block: in wT[:, col] (keys on partitions p,
                    # queries on cols) the keep condition is
                    # col - p - 128j >= 0, so cols [0,128j) are fully
                    # masked. Skip them in the scores matmul, the exp and
                    # the AV matmul; only [128j, 128j+128) needs the
                    # triangular select.
                    z = 128 * j if j > 0 else 0
                    sc = ps.tile([128, 2, 512], F32, tag="sc", bufs=2,
                                 name="sc")
                    for hh in range(2):
                        nc.tensor.matmul(
                            sc[:, hh, z:512],
                            kT[g][hh * 64:hh * 64 + 64,
                                  kt * 128:(kt + 1) * 128],
                            qT[g][hh * 64:hh * 64 + 64,
                                  q0 + z:q0 + 512],
                            start=True, stop=True,
                            tile_position=(64 * hh, 0),
                        )
                    wT = work.tile([128, 2, 512], BF, tag="wT", bufs=4)
                    if j >= 2:
                        # single strided instructions cover both heads'
                        # live columns: no per-half ACT overhead.
                        nc.vector.memset(wT[:, :, 0:z], 0.0)
                        nc.scalar.activation(
                            wT[:, :, z:512], sc[:, :, z:512],
                            EXP, scale=SCALE)
                        nc.gpsimd.affine_select(
                            out=wT[:, :, z:z + 128],
                            in_=wT[:, :, z:z + 128],
                            compare_op=mybir.AluOpType.is_ge,
                            fill=0.0,
                            base=0,
                            pattern=[[0, 2], [1, 128]],
                            channel_multiplier=-1,
                        )
                    elif j >= 0:
                        # full-width exp (cols [0,128j) hold stale psum,
                        # exp'd then zeroed by the select below)
                        nc.scalar.activation(wT, sc, EXP, scale=SCALE)
                        ncols = 128 * (j + 1)
                        nc.gpsimd.affine_select(
                            out=wT[:, :, 0:ncols],
                            in_=wT[:, :, 0:ncols],
                            compare_op=mybir.AluOpType.is_ge,
                            fill=0.0,
                            base=-z,
                            pattern=[[0, 2], [1, ncols]],
                            channel_multiplier=-1,
                        )
                    else:
                        nc.scalar.activation(wT, sc, EXP, scale=SCALE)
                    for hh, av in ((0, av0), (1, av1)):
                        nc.tensor.matmul(
                            av[:, z:512], V[:, kt, 2 * g + hh, :],
                            wT[:, hh, z:512],
                            start=(kt == 0), stop=(kt == nkt - 1),
                        )

                # ---- normalize: denominators via [128,8] reciprocal ----
                avc = work.tile([65, 1024], F32, tag="avc", bufs=2,
                                name="avc")
                nc.vector.tensor_copy(avc[:, 0:512], av0)
                nc.vector.tensor_copy(avc[:, 512:1024], av1)
                # SBUF->SBUF partition scatter of the denominator row so
                # the reciprocal runs on all 128 DVE lanes.
                # the very last group's chain runs after all exps are done,
                # so it can use the scalar ring and skip the sync-ring
                # backlog of the earlier groups' chains.
                neng = nc.scalar if (qb == 3 and g == 3) else nc.sync
                dsc = work.tile([128, 8], F32, tag="dsc", bufs=2, name="dsc")
                neng.dma_start(out=dsc, in_=avc[64:65, :])
                rec = work.tile([128, 8], F32, tag="rec", bufs=2, name="rec")
                nc.vector.reciprocal(rec, dsc)
                # gather back to DRAM in q-major order per head, then one
                # stride-0 broadcast read for both heads.
                neng.dma_start(
                    out=bass.AP(rrec_d[g].tensor, rrec_d[g].offset,
                                [[8, 128], [1, 8]]),
                    in_=rec)
                rep = work.tile([64, 1024], F32, tag="rep", bufs=2,
                                name="rep")
                neng.dma_start(
                    out=rep,
                    in_=bass.AP(rrec_d[g].tensor, rrec_d[g].offset,
                                [[0, 64], [1, 1024]]))
                nc.vector.tensor_mul(att[g][0:64, :], avc[0:64, 0:512],
                                     rep[:, 0:512])
                tmpB = work.tile([64, 512], BF, tag="tmpB", bufs=2,
                                 name="tmpB")
                nc.vector.tensor_mul(tmpB, avc[0:64, 512:1024],
                                     rep[:, 512:1024])
                neng.dma_start(out=att[g][64:128, :], in_=tmpB)

        # quarters 0-2 out-projections: emitted last (lowest priority) so
        # they act as pure filler for exp-paced attention phases.
        for pqb in range(3):
            emit_outproj(pqb, att_q[pqb])

        # quarter 3's out-projection forms the tail: use the sc PSUM
        # banks (free once the last scores ran) as paired accumulators so
        # two qtl are in flight and each drains with a single copy.
        for qtl in range(4):
            pspair = ps.tile([128, 2, 512], F32, tag="sc", bufs=2,
                             name="psyt")
            for half in range(2):
                for g in range(NG):
                    nc.tensor.matmul(
                        pspair[:, half, :],
                        att_q[3][g][:, qtl * 128:(qtl + 1) * 128],
                        wo_bf[:, g, half * 512:(half + 1) * 512],
                        start=(g == 0), stop=(g == NG - 1),
                    )
            y_sb = work.tile([128, C], F32, tag="y_sb", bufs=2, name="y_sb")
            nc.vector.tensor_copy(y_sb, pspair)
            nc.scalar.dma_start(
                out=y_d.ap()[(12 + qtl) * 128:(13 + qtl) * 128, :], in_=y_sb)


    nc.compile()
    return nc


_NC_CACHE = None


def _get_nc():
    global _NC_CACHE
    if _NC_CACHE is None:
        _NC_CACHE = build_nc()
    return _NC_CACHE


def kernel(x, w_qkv, w_out, _trace=False):
    import ml_dtypes

    bf16 = ml_dtypes.bfloat16
    B = x.shape[0]
    x = np.asarray(x, dtype=np.float32).astype(bf16)
    w_qkv = np.asarray(w_qkv, dtype=np.float32).astype(bf16)
    w_out = np.asarray(w_out, dtype=np.float32).astype(bf16)

    nc = _get_nc()
    in_maps = []
    for core in range(8):
        b = core % B
        hbase = (core // B) * HC
        lo, hi = hbase * D, hbase * D + HC * D
        in_maps.append({
            "x": np.ascontiguousarray(x[b].T),
            "wq": np.ascontiguousarray(w_qkv[:, lo:hi]),
            "wk": np.ascontiguousarray(w_qkv[:, C + lo:C + hi]),
            "wv": np.ascontiguousarray(w_qkv[:, 2 * C + lo:2 * C + hi]),
            "wo": np.ascontiguousarray(w_out[lo:hi, :]),
        })

    res = run_bass_kernel_spmd(nc, in_maps, core_ids=list(range(8)), trace=_trace)
    ys = [r["y"] for r in res.results]
    out = np.empty((B, T, C), dtype=np.float32)
    for b in range(B):
        out[b] = ys[b] + ys[b + B]
    if _trace:
        return out, res
    return out


# revision 54
# speedup vs baseline: 1.0971x; 1.0055x over previous
"""Causal self-attention for trn2, 8 NeuronCores.

Problem: x[4,2048,1024] @ w_qkv[1024,3072] -> causal MHA (16 heads, d=64)
-> @ w_out[1024,1024].

Sharding: core c handles batch b=c%4 and heads hbase=8*(c//4)..hbase+8
(data parallel on B x tensor parallel on heads). Each core computes the
partial out-projection y_c = att_slice @ w_out[slice]; the host sums the
two partials per batch.

Final design (v18, ~295us vs 519us baseline):
- Inputs arrive host-side pre-cast to bf16, with x pre-TRANSPOSED
  ([C,T]) so xT loads as plain DMAs into persistent SBUF tiles - no
  on-chip transposes at all.
- Per quarter, per head-group: q/k projection chains, then that group's
  attention immediately (early exps); V projection emitted before the
  first group's attention (program order defines dataflow for the
  diagonal AV reads).
- Scores for the two heads of a group are row-tiled (K=64 each,
  tile_position (0,0)/(64,0)) and run CONCURRENTLY into one [128,1024]
  PSUM pair; a single ScalarE exp covers both heads. The kernel is
  exp-paced (~1.12us per k-tile); projection/out-projection matmuls
  fill the TensorE gaps via the Tile scheduler.
- Softmax denominators come from the fused ones-column in AV (row 64 of
  the [65,512] accumulators). Normalization scatters the denominator
  row into a [128,8] tile so the reciprocal runs on all 128 DVE lanes,
  then DMA-broadcasts the reciprocals back via a DRAM bounce.
- Diagonal k-tiles: fully-masked leading columns are skipped in the
  scores matmul, exp and AV (N-trim); only a 128-wide band needs the
  gpsimd triangular select.
- DMA rings: sync = xT + normalization chains (engine-blocking waits are
  free there); scalar = weights + tail-quarter y (doorbells cost ~0.6us
  of issuing-engine time -> never put dep-waiting DMAs on scalar, which
  paces the exps); gpsimd SWDGE = bulk y writes.
- Out-projections for quarters 0-2 are emitted last (lowest priority,
  pure filler); quarter 3's uses the freed sc PSUM banks as paired
  accumulators to shorten the tail.
- PSUM: sc [128,1024]x2 (4 banks) + av0/av1 [65,512] (2 banks) +
  pj [128,512]x2 (2 banks) shared by qkv-proj, V-proj and out-proj.
"""

import sys

for p in ("/opt/trn_rl_repo", "/opt/pypackages"):
    if p not in sys.path:
        sys.path.insert(0, p)

import contextlib

import numpy as np

import concourse.bass as bass
import concourse.mybir as mybir
import concourse.tile as tile
from concourse import bacc
from concourse.bass_utils import run_bass_kernel_spmd
from concourse.masks import make_identity

F32 = mybir.dt.float32
BF = mybir.dt.bfloat16
EXP = mybir.ActivationFunctionType.Exp

T = 2048          # sequence length
C = 1024          # model dim
HC = 8            # heads per core
D = 64            # head dim
NG = 4            # head-groups of 2 per core
NCT = C // 128    # 8 contraction tiles
NTT = T // 128    # 16 token tiles
NQ = 4            # T quarters
SCALE = 0.125     # 1/sqrt(D)


def build_nc():
    nc = bacc.Bacc("TRN2", target_bir_lowering=False, debug=False)

    # All matmul operands are bf16 anyway, so inputs arrive pre-cast to
    # bf16 from the host: halves the startup DMA bytes and removes every
    # staging cast (device cast via DVE would be identical numerics).
    # x additionally arrives PRE-TRANSPOSED ([C, T]) so the xT operand the
    # projections need loads with plain DMAs - no on-chip transposes.
    x_d = nc.dram_tensor("x", [C, T], BF, kind="ExternalInput")
    wq_d = nc.dram_tensor("wq", [C, 512], BF, kind="ExternalInput")
    wk_d = nc.dram_tensor("wk", [C, 512], BF, kind="ExternalInput")
    wv_d = nc.dram_tensor("wv", [C, 512], BF, kind="ExternalInput")
    wo_d = nc.dram_tensor("wo", [512, C], BF, kind="ExternalInput")
    y_d = nc.dram_tensor("y", [T, C], F32, kind="ExternalOutput")

    with tile.TileContext(nc) as tc, contextlib.ExitStack() as ctx:
        persist = ctx.enter_context(tc.tile_pool(name="persist", bufs=1))
        work = ctx.enter_context(tc.tile_pool(name="work", bufs=1))
        ps = ctx.enter_context(tc.tile_pool(name="ps", bufs=1, space="PSUM"))
        dpool = ctx.enter_context(tc.tile_pool(name="dram", bufs=1, space="DRAM"))

        kT = [persist.tile([128, T], BF, tag=f"kT{g}", name=f"kT{g}")
              for g in range(NG)]
        qT = [persist.tile([128, T], BF, tag=f"qT{g}", name=f"qT{g}")
              for g in range(NG)]
        V = persist.tile([128, NTT, HC, 65], BF, tag="V")

        # xT resident in SBUF: one tile PER QUARTER (separate tiles keep
        # the dependency tracking fine-grained, so quarter-0 projections
        # never wait on later quarters' loads).
        xT_q = [persist.tile([128, NCT, 512], BF, tag=f"xTq{rnd}",
                              name=f"xTq{rnd}")
                for rnd in range(NQ)]

        def load_xT(rnd):
            nc.sync.dma_start(
                out=xT_q[rnd],
                in_=x_d.ap()[:, rnd * 512:(rnd + 1) * 512].rearrange(
                    "(ct p) t -> p ct t", p=128))

        for rnd in range(NQ):
            load_xT(rnd)

        # weights: bf16 loads on the scalar HWDGE queue; the sync queue
        # carries xT and the normalization bounce.
        wq_bf = persist.tile([128, NCT, 512], BF, tag="wq_bf")
        wk_bf = persist.tile([128, NCT, 512], BF, tag="wk_bf")
        wv_bf = persist.tile([128, NCT, 512], BF, tag="wv_bf")
        wo_bf = persist.tile([128, NG, C], BF, tag="wo_bf")
        for wdram, wbf in ((wq_d, wq_bf), (wk_d, wk_bf), (wv_d, wv_bf)):
            nc.scalar.dma_start(
                out=wbf, in_=wdram.ap().rearrange("(ct p) m -> p ct m", p=128))
        nc.scalar.dma_start(
            out=wo_bf, in_=wo_d.ap().rearrange("(g p) c -> p g c", p=128))

        # ones column of V (fused softmax denominator)
        ones_f32 = persist.tile([128, NTT, HC], F32, tag="ones")
        nc.vector.memset(ones_f32, 1.0)
        nc.vector.tensor_copy(V[:, :, :, 64], ones_f32)

        # DRAM scratch for the reciprocal broadcast bounce
        rrec_d = [dpool.tile([1, 1024], F32, tag=f"rrec{i}", name=f"rrec{i}",
                             bufs=2)
                  for i in range(NG)]

        def emit_outproj(qb, att):
            # out projection for quarter qb's q rows. Emitted AFTER the
            # next quarter's projections so the shared "pj" PSUM rotation
            # never makes projections wait on the normalization chain.
            for qtl in range(4):
                qt = qb * 4 + qtl
                y_sb = work.tile([128, C], F32, tag="y_sb", bufs=2,
                                 name="y_sb")
                for half in range(2):
                    psy = ps.tile([128, 512], F32, tag="pj", bufs=2,
                                  name="psy")
                    for g in range(NG):
                        nc.tensor.matmul(
                            psy,
                            att[g][:, qtl * 128:(qtl + 1) * 128],
                            wo_bf[:, g, half * 512:(half + 1) * 512],
                            start=(g == 0),
                            stop=(g == NG - 1),
                        )
                    nc.vector.tensor_copy(
                        y_sb[:, half * 512:(half + 1) * 512], psy)
                # y rides the otherwise-idle gpsimd SWDGE ring, except
                # the final quarter which uses the (by then idle) scalar
                # ring for lower tail latency.
                yeng = nc.scalar if qb == 3 else nc.gpsimd
                yeng.dma_start(
                    out=y_d.ap()[qt * 128:(qt + 1) * 128, :], in_=y_sb
                )

        def emit_V(vq):
            # V projection for quarter vq. Must be emitted BEFORE that
            # quarter's attention (program order defines dataflow: the
            # diagonal AVs read it). Quarter vq+1's V is emitted at the
            # END of section vq so a quarter boundary only waits on the
            # next group's q/k chain, never on V.
            for tt in range(4):
                pv = ps.tile([128, HC, 64], F32, tag="pj", bufs=2,
                             name="pv")
                for ct in range(NCT):
                    nc.tensor.matmul(
                        pv,
                        xT_q[vq][:, ct, tt * 128:(tt + 1) * 128],
                        wv_bf[:, ct, :],
                        start=(ct == 0), stop=(ct == NCT - 1),
                    )
                nc.vector.tensor_copy(V[:, vq * 4 + tt, :, 0:64], pv)

        att_q = {}
        for qb in range(NQ):
            q0 = qb * 512
            nkt = 4 * (qb + 1)
            att = [work.tile([128, 512], BF, tag=f"att{g}", name=f"att{g}",
                             bufs=4)
                   for g in range(NG)]
            att_q[qb] = att
            for g in range(NG):
                # ---- qT/kT for group g ----
                for which, wbf, dst in ((0, wq_bf, qT[g]), (1, wk_bf, kT[g])):
                    pj = ps.tile([128, 512], F32, tag="pj", bufs=2, name="pj")
                    for ct in range(NCT):
                        nc.tensor.matmul(
                            pj,
                            wbf[:, ct, g * 128:(g + 1) * 128],
                            xT_q[qb][:, ct, :],
                            start=(ct == 0), stop=(ct == NCT - 1),
                        )
                    nc.vector.tensor_copy(dst[:, q0:q0 + 512], pj)

                if g == 0 and qb > 0:
                    emit_V(qb)

                # ---- attention for group g ----
                av0 = ps.tile([65, 512], F32, tag="av0", name="av0")
                av1 = ps.tile([65, 512], F32, tag="av1", name="av1")
                wt_defer = []
                for kt in range(nkt):
                    j = kt - 4 * qb
                    # diagonal block: in wT[:, col] (keys on partitions p,
                    # queries on cols) the keep condition is
                    # col - p - 128j >= 0, so cols [0,128j) are fully
                    # masked. Skip them in the scores matmul, the exp and
                    # the AV matmul; only [128j, 128j+128) needs the
                    # triangular select.
                    z = 128 * j if j > 0 else 0
                    sc = ps.tile([128, 2, 512], F32, tag="sc", bufs=2,
                                 name="sc")
                    for hh in range(2):
                        nc.tensor.matmul(
                            sc[:, hh, z:512],
                            kT[g][hh * 64:hh * 64 + 64,
                                  kt * 128:(kt + 1) * 128],
                            qT[g][hh * 64:hh * 64 + 64,
                                  q0 + z:q0 + 512],
                            start=True, stop=True,
                            tile_position=(64 * hh, 0),
                        )
                    wT = work.tile([128, 2, 512], BF, tag="wT", bufs=4)
                    if j >= 2:
                        # single strided instructions cover both heads'
                        # live columns: no per-half ACT overhead.
                        nc.vector.memset(wT[:, :, 0:z], 0.0)
                        nc.scalar.activation(
                            wT[:, :, z:512], sc[:, :, z:512],
                            EXP, scale=SCALE)
                        nc.gpsimd.affine_select(
                            out=wT[:, :, z:z + 128],
                            in_=wT[:, :, z:z + 128],
                            compare_op=mybir.AluOpType.is_ge,
                            fill=0.0,
                            base=0,
                            pattern=[[0, 2], [1, 128]],
                            channel_multiplier=-1,
                        )
                    elif j >= 0:
                        # full-width exp (cols [0,128j) hold stale psum,
                        # exp'd then zeroed by the select below)
                        nc.scalar.activation(wT, sc, EXP, scale=SCALE)
                        ncols = 128 * (j + 1)
                        nc.gpsimd.affine_select(
                            out=wT[:, :, 0:ncols],
                            in_=wT[:, :, 0:ncols],
                            compare_op=mybir.AluOpType.is_ge,
                            fill=0.0,
                            base=-z,
                            pattern=[[0, 2], [1, ncols]],
                            channel_multiplier=-1,
                        )
                    else:
                        nc.scalar.activation(wT, sc, EXP, scale=SCALE)
                    if qb == 0 and g == 0:
                        # defer the AVs: lets the first exps run before
                        # the V projection (which still needs wv from HBM)
                        wt_defer.append((wT, z))
                    else:
                        for hh, av in ((0, av0), (1, av1)):
                            nc.tensor.matmul(
                                av[:, z:512], V[:, kt, 2 * g + hh, :],
                                wT[:, hh, z:512],
                                start=(kt == 0), stop=(kt == nkt - 1),
                            )

                if qb == 0 and g == 0:
                    emit_V(0)
                    for dkt, (wTd, zd) in enumerate(wt_defer):
                        for hh, av in ((0, av0), (1, av1)):
                            nc.tensor.matmul(
                                av[:, zd:512], V[:, dkt, 2 * g + hh, :],
                                wTd[:, hh, zd:512],
                                start=(dkt == 0), stop=(dkt == nkt - 1),
                            )

                # ---- normalize: denominators via [128,8] reciprocal ----
                avc = work.tile([65, 1024], F32, tag="avc", bufs=2,
                                name="avc")
                nc.vector.tensor_copy(avc[:, 0:512], av0)
                nc.vector.tensor_copy(avc[:, 512:1024], av1)
                # SBUF->SBUF partition scatter of the denominator row so
                # the reciprocal runs on all 128 DVE lanes.
                # the very last group's chain runs after all exps are done,
                # so it can use the scalar ring and skip the sync-ring
                # backlog of the earlier groups' chains.
                neng = nc.scalar if (qb == 3 and g == 3) else nc.sync
                dsc = work.tile([128, 8], F32, tag="dsc", bufs=2, name="dsc")
                neng.dma_start(out=dsc, in_=avc[64:65, :])
                rec = work.tile([128, 8], F32, tag="rec", bufs=2, name="rec")
                nc.vector.reciprocal(rec, dsc)
                # gather back to DRAM in q-major order per head, then one
                # stride-0 broadcast read for both heads.
                neng.dma_start(
                    out=bass.AP(rrec_d[g].tensor, rrec_d[g].offset,
                                [[8, 128], [1, 8]]),
                    in_=rec)
                rep = work.tile([64, 1024], F32, tag="rep", bufs=2,
                                name="rep")
                neng.dma_start(
                    out=rep,
                    in_=bass.AP(rrec_d[g].tensor, rrec_d[g].offset,
                                [[0, 64], [1, 1024]]))
                nc.vector.tensor_mul(att[g][0:64, :], avc[0:64, 0:512],
                                     rep[:, 0:512])
                tmpB = work.tile([64, 512], BF, tag="tmpB", bufs=2,
                                 name="tmpB")
                nc.vector.tensor_mul(tmpB, avc[0:64, 512:1024],
                                     rep[:, 512:1024])
                neng.dma_start(out=att[g][64:128, :], in_=tmpB)

        # quarters 0-2 out-projections: emitted last (lowest priority) so
        # they act as pure filler for exp-paced attention phases.
        for pqb in range(3):
            emit_outproj(pqb, att_q[pqb])

        # quarter 3's out-projection forms the tail: use the sc PSUM
        # banks (free once the last scores ran) as paired accumulators so
        # two qtl are in flight and each drains with a single copy.
        for qtl in range(4):
            pspair = ps.tile([128, 2, 512], F32, tag="sc", bufs=2,
                             name="psyt")
            for half in range(2):
                for g in range(NG):
                    nc.tensor.matmul(
                        pspair[:, half, :],
                        att_q[3][g][:, qtl * 128:(qtl + 1) * 128],
                        wo_bf[:, g, half * 512:(half + 1) * 512],
                        start=(g == 0), stop=(g == NG - 1),
                    )
            y_sb = work.tile([128, C], F32, tag="y_sb", bufs=2, name="y_sb")
            nc.vector.tensor_copy(y_sb, pspair)
            nc.scalar.dma_start(
                out=y_d.ap()[(12 + qtl) * 128:(13 + qtl) * 128, :], in_=y_sb)


    nc.compile()
    return nc


_NC_CACHE = None


def _get_nc():
    global _NC_CACHE
    if _NC_CACHE is None:
        _NC_CACHE = build_nc()
    return _NC_CACHE


def kernel(x, w_qkv, w_out, _trace=False):
    import ml_dtypes

    bf16 = ml_dtypes.bfloat16
    B = x.shape[0]
    x = np.asarray(x, dtype=np.float32).astype(bf16)
    w_qkv = np.asarray(w_qkv, dtype=np.float32).astype(bf16)
    w_out = np.asarray(w_out, dtype=np.float32).astype(bf16)

    nc = _get_nc()
    in_maps = []
    for core in range(8):
        b = core % B
        hbase = (core // B) * HC
        lo, hi = hbase * D, hbase * D + HC * D
        in_maps.append({
            "x": np.ascontiguousarray(x[b].T),
            "wq": np.ascontiguousarray(w_qkv[:, lo:hi]),
            "wk": np.ascontiguousarray(w_qkv[:, C + lo:C + hi]),
            "wv": np.ascontiguousarray(w_qkv[:, 2 * C + lo:2 * C + hi]),
            "wo": np.ascontiguousarray(w_out[lo:hi, :]),
        })

    res = run_bass_kernel_spmd(nc, in_maps, core_ids=list(range(8)), trace=_trace)
    ys = [r["y"] for r in res.results]
    out = np.empty((B, T, C), dtype=np.float32)
    for b in range(B):
        out[b] = ys[b] + ys[b + B]
    if _trace:
        return out, res
    return out
